# revision 22
# baseline (speedup 1.0000x reference)
"""CRF negative-log-likelihood loss on 8 Trainium2 NeuronCores.

Math (per batch row b, reference semantics, mask == all-ones):
    loss[b] = logsumexp_scan(logits[b], trans) - gold_score(logits[b], tags[b], trans)

Device algorithm (linear domain):
    E = exp(trans), g_t = exp(emit_t - kappa)
    alpha-exp recurrence:  q_t = (E^T q_{t-1}) * g_t          (a 64x64 matmul + eltwise mul)
    Z = v0 . (A_1 ... A_511) . 1  with A_t = E diag(g_t)  is split into a forward
    scan from t=0 and a backward scan from t=511 that meet in the middle, run
    concurrently on PE array quadrants (partitions 0-63 fwd / 64-127 bwd).
    Per-row sum-renormalization every KR steps keeps fp32 in range; log offsets
    are accumulated on device.  Partition-dim reductions/broadcasts are done as
    matmuls against ones vectors (no transposes anywhere).
    Gold emission score: one-hot tiles built on GPSIMD (partition-broadcast tag
    rows + is_equal vs a per-partition iota), accumulated against the raw
    emission tiles with PSUM-accumulating matmuls; the diagonal of the result
    is the per-row emission score.  Gold transition score: host-indexed
    trans[tag_t, tag_{t+1}] table (tiny tags-only preprocessing), reduced on
    device.

Sharding: pure data parallel, batch 1024 -> 8 cores x 128 rows.
"""
import os
import sys

import numpy as np

sys.path.insert(0, "/opt/trn_rl_repo")

from concourse import bacc, bass, mybir, tile  # noqa: E402
from concourse.bass_utils import run_bass_kernel_spmd  # noqa: E402

F32 = mybir.dt.float32
AF = mybir.ActivationFunctionType
ALU = mybir.AluOpType

B, L, T = 1024, 512, 64
NCORES = 8
BL = B // NCORES  # 128 batch rows per core
S = L // 2  # 256 fwd/bwd step-pairs
KR = 32  # renorm every KR pairs (fp32 range stays within ~1e8)
CHUNK = 8  # pairs per DMA/exp chunk -> [128, 1024] tiles
NCH = S // CHUNK

LAST_RESULTS = None  # BassKernelResults of the most recent run (for test harness)


def _build(kappa: float) -> bass.Bass:
    nc = bacc.Bacc("TRN2", target_bir_lowering=False, debug=False, num_devices=NCORES)
    # x layout: [chunk, partition, pair-within-chunk * b] — each chunk tile is
    # one contiguous-per-partition [128, CHUNK*BL] DMA.
    x = nc.dram_tensor("x", [NCH, 128, CHUNK * BL], F32, kind="ExternalInput")
    wstat = nc.dram_tensor("wstat", [128, T], F32, kind="ExternalInput")
    # per chunk: CHUNK*BL fwd tag values then CHUNK*BL bwd tag values (as f32)
    tagrows = nc.dram_tensor("tagrows", [NCH, 2 * CHUNK * BL], F32, kind="ExternalInput")
    io64 = nc.dram_tensor("io64", [128, 1], F32, kind="ExternalInput")  # p % 64
    ident = nc.dram_tensor("ident", [128, 128], F32, kind="ExternalInput")
    tvals = nc.dram_tensor("tvals", [BL, L], F32, kind="ExternalInput")
    out_logz = nc.dram_tensor("logz", [BL], F32, kind="ExternalOutput")
    out_gold = nc.dram_tensor("gold", [BL], F32, kind="ExternalOutput")

    with tile.TileContext(nc) as tc:
        with (
            tc.tile_pool(name="cpool", bufs=1) as cpool,
            tc.tile_pool(name="rawp", bufs=3) as rawp,
            tc.tile_pool(name="gp", bufs=3) as gp,
            tc.tile_pool(name="ohp", bufs=3) as ohp,
            tc.tile_pool(name="qp", bufs=4) as qp,
            tc.tile_pool(name="mp", bufs=2) as mp,
            tc.tile_pool(name="pp", bufs=3, space="PSUM") as pp,
            tc.tile_pool(name="pp2", bufs=2, space="PSUM") as pp2,
            tc.tile_pool(name="ppg", bufs=1, space="PSUM") as ppg,
        ):
            # ---- constants ----
            wsb = cpool.tile([128, T], F32)  # rows 0-63: E (fwd lhsT), 64-127: E^T (bwd lhsT)
            nc.sync.dma_start(out=wsb[:], in_=wstat[:])
            ones = cpool.tile([128, T], F32)
            nc.gpsimd.memset(ones[:], 1.0)
            stats = cpool.tile([128, BL], F32)  # row 64 = accumulated log-offsets o
            nc.vector.memset(stats[:], 0.0)
            o_acc = stats[64:65, :]
            kbias = cpool.tile([128, 1], F32)  # per-partition bias = -kappa for Exp
            nc.vector.memset(kbias[:], -kappa)
            iot = cpool.tile([128, 1], F32)
            nc.sync.dma_start(out=iot[:], in_=io64[:])
            idt = cpool.tile([128, 128], F32)
            nc.sync.dma_start(out=idt[:], in_=ident[:])
            tvt = cpool.tile([BL, L], F32)
            nc.sync.dma_start(out=tvt[:], in_=tvals[:])

            g2 = ppg.tile([128, 128], F32)  # gold-emission accumulator

            # ---- fwd/bwd scan + gold-emission accumulation ----
            qh = None  # [128, BL]: rows 0-63 fwd state q, rows 64-127 bwd state h
            pending = None  # (apply_at_pair, R2 psum tile) for lagged renorm
            for c in range(NCH):
                raw = rawp.tile([128, CHUNK * BL], F32)
                nc.sync.dma_start(out=raw[:], in_=x[c])
                g = gp.tile([128, CHUNK * BL], F32)
                nc.scalar.activation(g[:], raw[:], AF.Exp, bias=kbias[:])
                # one-hot tiles for the gold emission term
                tagr = mp.tile([1, 2 * CHUNK * BL], F32, tag="tagr")
                nc.sync.dma_start(out=tagr[:], in_=tagrows[c].unsqueeze(0))
                tagbc = ohp.tile([128, 2 * CHUNK * BL], F32, tag="tagbc")
                nc.gpsimd.partition_broadcast(tagbc[:], tagr[:], channels=128)
                oh = ohp.tile([128, CHUNK * BL], F32, tag="oh")
                nc.gpsimd.tensor_scalar(
                    oh[0:64, :], tagbc[0:64, 0 : CHUNK * BL], iot[0:64, :], None, ALU.is_equal
                )
                nc.gpsimd.tensor_scalar(
                    oh[64:128, :], tagbc[64:128, CHUNK * BL :], iot[64:128, :], None, ALU.is_equal
                )
                for k in range(CHUNK):
                    s = c * CHUNK + k
                    sl = slice(k * BL, (k + 1) * BL)
                    nc.tensor.matmul(
                        out=g2[:],
                        lhsT=oh[:, sl],
                        rhs=raw[:, sl],
                        start=(s == 0),
                        stop=(s == S - 1),
                        skip_group_check=True,
                    )
                    gt = g[:, sl]
                    if s == 0:
                        qh = gt  # q_0 = g_0 (fwd), h_511 = g_511 (bwd)
                        continue
                    ps = pp.tile([128, BL], F32, tag="ps")
                    nc.tensor.matmul(out=ps[0:64, :], lhsT=wsb[0:64, :], rhs=qh[0:64, :])
                    nc.tensor.matmul(out=ps[64:128, :], lhsT=wsb[64:128, :], rhs=qh[64:128, :])
                    nqh = qp.tile([128, BL], F32, tag="qh")
                    nc.vector.tensor_tensor(out=nqh[:], in0=ps[:], in1=gt, op=ALU.mult)
                    qh = nqh
                    if pending is not None and pending[0] == s:
                        r2 = pending[1]
                        nqh2 = qp.tile([128, BL], F32, tag="qh")
                        nc.vector.tensor_tensor(out=nqh2[:], in0=qh[:], in1=r2[:], op=ALU.mult)
                        qh = nqh2
                        pending = None
                    if s % KR == 0 and s + 2 < S:
                        # colsums of both chains (partition-dim reduction via ones matmul)
                        rcf = pp2.tile([128, BL], F32, tag="rc")
                        rcb = pp2.tile([128, BL], F32, tag="rc")
                        nc.tensor.matmul(out=rcf[64:65, :], lhsT=ones[0:64, 0:1], rhs=qh[0:64, :])
                        nc.tensor.matmul(out=rcb[64:65, :], lhsT=ones[64:128, 0:1], rhs=qh[64:128, :])
                        # o += log rf + log rb
                        lr = mp.tile([128, BL], F32, tag="lr")
                        nc.scalar.activation(lr[64:65, :], rcf[64:65, :], AF.Ln)
                        nc.vector.tensor_add(out=o_acc, in0=o_acc, in1=lr[64:65, :])
                        lr2 = mp.tile([128, BL], F32, tag="lr")
                        nc.scalar.activation(lr2[64:65, :], rcb[64:65, :], AF.Ln)
                        nc.vector.tensor_add(out=o_acc, in0=o_acc, in1=lr2[64:65, :])
                        # 1/r, broadcast to 64 partitions via K=1 ones matmul
                        rinv = mp.tile([128, BL], F32, tag="rinv")
                        nc.vector.reciprocal(out=rinv[64:65, :], in_=rcf[64:65, :])
                        rinv2 = mp.tile([128, BL], F32, tag="rinv")
                        nc.vector.reciprocal(out=rinv2[64:65, :], in_=rcb[64:65, :])
                        r2t = pp2.tile([128, BL], F32, tag="r2", bufs=1)
                        nc.tensor.matmul(out=r2t[0:64, :], lhsT=ones[64:65, 0:64], rhs=rinv[64:65, :])
                        nc.tensor.matmul(out=r2t[64:128, :], lhsT=ones[64:65, 0:64], rhs=rinv2[64:65, :])
                        pending = (s + 2, r2t)

            # ---- final combine: Z = q_255 . (E h_256) ----
            psf = pp.tile([128, BL], F32, tag="ps")
            nc.tensor.matmul(
                out=psf[0:64, :], lhsT=wsb[64:128, :], rhs=qh[64:128, :], tile_position=(64, 0)
            )
            ztmp = mp.tile([128, BL], F32)
            nc.vector.tensor_tensor(out=ztmp[0:64, :], in0=psf[0:64, :], in1=qh[0:64, :], op=ALU.mult)
            zc = pp2.tile([128, BL], F32, tag="rc")
            nc.tensor.matmul(out=zc[64:65, :], lhsT=ones[0:64, 0:1], rhs=ztmp[0:64, :])
            lz = mp.tile([128, BL], F32)
            nc.scalar.activation(lz[64:65, :], zc[64:65, :], AF.Ln)
            nc.vector.tensor_add(out=lz[64:65, :], in0=lz[64:65, :], in1=o_acc)
            nc.sync.dma_start(out=out_logz[:], in_=lz[64:65, :])

            # ---- gold total: diag(G2) + sum(tvals) ----
            dtile = cpool.tile([128, 128], F32)
            nc.vector.tensor_tensor(out=dtile[:], in0=g2[:], in1=idt[:], op=ALU.mult)
            gold_e = cpool.tile([BL, 1], F32)
            nc.vector.reduce_sum(gold_e[:], dtile[:], axis=mybir.AxisListType.X)
            gold_t = cpool.tile([BL, 1], F32)
            nc.vector.reduce_sum(gold_t[:], tvt[:], axis=mybir.AxisListType.X)
            gold = cpool.tile([BL, 1], F32)
            nc.vector.tensor_add(out=gold[:], in0=gold_e[:], in1=gold_t[:])
            nc.sync.dma_start(out=out_gold[:], in_=gold[:, 0:1])
    nc.finalize()
    return nc


def kernel(**inputs) -> np.ndarray:
    global LAST_RESULTS
    logits = np.asarray(inputs["logits"], dtype=np.float32)  # [1024, 512, 64]
    tags = np.asarray(inputs["tags"]).astype(np.int64)  # [1024, 512]
    trans = np.asarray(inputs["trans_m"], dtype=np.float32)  # [64, 64]
    # mask is all-ones by construction (spec fill=ones); under all-ones the
    # reference's mask terms are identities, so it is not used here.

    E = np.exp(trans).astype(np.float32)
    kappa = float(np.log(np.exp(trans.astype(np.float64)).sum(axis=0)).mean())
    wstat = np.concatenate([E, E.T], axis=0).astype(np.float32)  # [128, 64]
    io64 = (np.arange(128) % T).astype(np.float32).reshape(128, 1)
    ident = np.eye(128, dtype=np.float32)

    nc = _build(kappa)

    in_maps = []
    for c in range(NCORES):
        sh = logits[c * BL : (c + 1) * BL]  # [128, 512, 64]
        xt = sh.transpose(1, 2, 0)  # [t, j, b]
        x2 = np.concatenate([xt[0:S], xt[S:L][::-1]], axis=1)  # [256, 128, 128] = [s, p, b]
        # regroup into [chunk, partition, pair-in-chunk*b] contiguous chunks
        x3 = (
            x2.reshape(NCH, CHUNK, 128, BL)
            .transpose(0, 2, 1, 3)
            .reshape(NCH, 128, CHUNK * BL)
        )
        tg = tags[c * BL : (c + 1) * BL]  # [128, 512]
        # tag rows per chunk: fwd tags tag[b, s] then bwd tags tag[b, 511-s], s-major
        tgf = tg.T[:S]  # [s, b] for t = s
        tgb = tg.T[::-1][:S]  # [s, b] for t = 511 - s
        tagrows = np.concatenate(
            [tgf.reshape(NCH, CHUNK * BL), tgb.reshape(NCH, CHUNK * BL)], axis=1
        ).astype(np.float32)  # [NCH, 2*CHUNK*BL]
        tvals = np.zeros((BL, L), np.float32)
        tvals[:, : L - 1] = trans[tg[:, :-1], tg[:, 1:]]
        in_maps.append(
            {
                "x": np.ascontiguousarray(x3, dtype=np.float32),
                "wstat": wstat,
                "tagrows": tagrows,
                "io64": io64,
                "ident": ident,
                "tvals": tvals,
            }
        )

    res = run_bass_kernel_spmd(
        nc,
        in_maps,
        list(range(NCORES)),
        trace=bool(int(os.environ.get("CRF_TRACE", "0"))),
    )
    LAST_RESULTS = res

    out = np.empty((B,), np.float32)
    kc = np.float32(L * kappa)
    for c in range(NCORES):
        logz = np.asarray(res.results[c]["logz"], np.float32).reshape(BL)
        gold = np.asarray(res.results[c]["gold"], np.float32).reshape(BL)
        out[c * BL : (c + 1) * BL] = logz + kc - gold
    return out


# revision 23
# speedup vs baseline: 3.0157x; 3.0157x over previous
"""CRF negative-log-likelihood loss on 8 Trainium2 NeuronCores.

Math (per batch row b, reference semantics, mask == all-ones):
    loss[b] = logsumexp_scan(logits[b], trans) - gold_score(logits[b], tags[b], trans)

Device algorithm (linear domain):
    E = exp(trans), g_t = exp(emit_t - kappa)
    alpha-exp recurrence:  q_t = (E^T q_{t-1}) * g_t          (a 64x64 matmul + eltwise mul)
    Z = v0 . (A_1 ... A_511) . 1  with A_t = E diag(g_t)  is split into a forward
    scan from t=0 and a backward scan from t=511 that meet in the middle, run
    concurrently on PE array quadrants (partitions 0-63 fwd / 64-127 bwd).
    Per-row sum-renormalization every KR steps keeps fp32 in range; log offsets
    are accumulated on device.  Partition-dim reductions/broadcasts are done as
    matmuls against ones vectors (no transposes anywhere).
    Gold emission score: one-hot tiles built on GPSIMD (partition-broadcast tag
    rows + is_equal vs a per-partition iota), accumulated against the raw
    emission tiles with PSUM-accumulating matmuls; the diagonal of the result
    is the per-row emission score.  Gold transition score: host-indexed
    trans[tag_t, tag_{t+1}] table (tiny tags-only preprocessing), reduced on
    device.

Sharding: pure data parallel, batch 1024 -> 8 cores x 128 rows.
"""
import os
import sys

import numpy as np

sys.path.insert(0, "/opt/trn_rl_repo")

from concourse import bacc, bass, mybir, tile  # noqa: E402
from concourse.bass_utils import run_bass_kernel_spmd  # noqa: E402

F32 = mybir.dt.float32
U8 = mybir.dt.uint8
AF = mybir.ActivationFunctionType
ALU = mybir.AluOpType

B, L, T = 1024, 512, 64
NCORES = 8
BL = B // NCORES  # 128 batch rows per core
S = L // 2  # 256 fwd/bwd step-pairs
KR = 32  # renorm every KR pairs (fp32 range stays within ~1e8)
CHUNK = 8  # pairs per DMA/exp chunk -> [128, 1024] tiles
NCH = S // CHUNK

LAST_RESULTS = None  # BassKernelResults of the most recent run (for test harness)


def _build(kappa: float) -> bass.Bass:
    nc = bacc.Bacc("TRN2", target_bir_lowering=False, debug=False, num_devices=NCORES)
    # x layout: [chunk, partition, pair-within-chunk * b] — each chunk tile is
    # one contiguous-per-partition [128, CHUNK*BL] DMA.
    x = nc.dram_tensor("x", [NCH, 128, CHUNK * BL], F32, kind="ExternalInput")
    wstat = nc.dram_tensor("wstat", [128, T], F32, kind="ExternalInput")
    # per chunk: one-hot of the gold tag per (partition-half, pair, b), as u8
    ohu = nc.dram_tensor("ohu", [NCH, 128, CHUNK * BL], U8, kind="ExternalInput")
    ident = nc.dram_tensor("ident", [128, 128], F32, kind="ExternalInput")
    tvals = nc.dram_tensor("tvals", [BL, L], F32, kind="ExternalInput")
    out_logz = nc.dram_tensor("logz", [BL], F32, kind="ExternalOutput")
    out_gold = nc.dram_tensor("gold", [BL], F32, kind="ExternalOutput")

    with tile.TileContext(nc) as tc:
        with (
            tc.tile_pool(name="cpool", bufs=1) as cpool,
            tc.tile_pool(name="rawp", bufs=3) as rawp,
            tc.tile_pool(name="gp", bufs=3) as gp,
            tc.tile_pool(name="ohp", bufs=3) as ohp,
            tc.tile_pool(name="qp", bufs=4) as qp,
            tc.tile_pool(name="mp", bufs=2) as mp,
            tc.tile_pool(name="pp", bufs=3, space="PSUM") as pp,
            tc.tile_pool(name="pp2", bufs=2, space="PSUM") as pp2,
            tc.tile_pool(name="ppg", bufs=1, space="PSUM") as ppg,
        ):
            # ---- constants ----
            wsb = cpool.tile([128, T], F32)  # rows 0-63: E (fwd lhsT), 64-127: E^T (bwd lhsT)
            nc.sync.dma_start(out=wsb[:], in_=wstat[:])
            ones = cpool.tile([128, T], F32)
            nc.gpsimd.memset(ones[:], 1.0)
            stats = cpool.tile([128, BL], F32)  # row 64 = accumulated log-offsets o
            nc.vector.memset(stats[:], 0.0)
            o_acc = stats[64:65, :]
            kbias = cpool.tile([128, 1], F32)  # per-partition bias = -kappa for Exp
            nc.vector.memset(kbias[:], -kappa)
            idt = cpool.tile([128, 128], F32)
            nc.sync.dma_start(out=idt[:], in_=ident[:])
            tvt = cpool.tile([BL, L], F32)
            nc.sync.dma_start(out=tvt[:], in_=tvals[:])

            g2 = ppg.tile([128, 128], F32)  # gold-emission accumulator

            # ---- fwd/bwd scan + gold-emission accumulation ----
            qh = None  # [128, BL]: rows 0-63 fwd state q, rows 64-127 bwd state h
            pending = None  # (apply_at_pair, R2 psum tile) for lagged renorm
            for c in range(NCH):
                raw = rawp.tile([128, CHUNK * BL], F32)
                nc.sync.dma_start(out=raw[:], in_=x[c])
                g = gp.tile([128, CHUNK * BL], F32)
                nc.scalar.activation(g[:], raw[:], AF.Exp, bias=kbias[:])
                # one-hot tiles for the gold emission term: u8 upload -> f32
                oh8 = ohp.tile([128, CHUNK * BL], U8, tag="oh8")
                nc.sync.dma_start(out=oh8[:], in_=ohu[c])
                oh = ohp.tile([128, CHUNK * BL], F32, tag="oh")
                nc.vector.tensor_copy(out=oh[:], in_=oh8[:])
                for k in range(CHUNK):
                    s = c * CHUNK + k
                    sl = slice(k * BL, (k + 1) * BL)
                    nc.tensor.matmul(
                        out=g2[:],
                        lhsT=oh[:, sl],
                        rhs=raw[:, sl],
                        start=(s == 0),
                        stop=(s == S - 1),
                        skip_group_check=True,
                    )
                    gt = g[:, sl]
                    if s == 0:
                        qh = gt  # q_0 = g_0 (fwd), h_511 = g_511 (bwd)
                        continue
                    ps = pp.tile([128, BL], F32, tag="ps")
                    nc.tensor.matmul(out=ps[0:64, :], lhsT=wsb[0:64, :], rhs=qh[0:64, :])
                    nc.tensor.matmul(out=ps[64:128, :], lhsT=wsb[64:128, :], rhs=qh[64:128, :])
                    nqh = qp.tile([128, BL], F32, tag="qh")
                    nc.vector.tensor_tensor(out=nqh[:], in0=ps[:], in1=gt, op=ALU.mult)
                    qh = nqh
                    if pending is not None and pending[0] == s:
                        r2 = pending[1]
                        nqh2 = qp.tile([128, BL], F32, tag="qh")
                        nc.vector.tensor_tensor(out=nqh2[:], in0=qh[:], in1=r2[:], op=ALU.mult)
                        qh = nqh2
                        pending = None
                    if s % KR == 0 and s + 2 < S:
                        # colsums of both chains (partition-dim reduction via ones matmul)
                        rcf = pp2.tile([128, BL], F32, tag="rc")
                        rcb = pp2.tile([128, BL], F32, tag="rc")
                        nc.tensor.matmul(out=rcf[64:65, :], lhsT=ones[0:64, 0:1], rhs=qh[0:64, :])
                        nc.tensor.matmul(out=rcb[64:65, :], lhsT=ones[64:128, 0:1], rhs=qh[64:128, :])
                        # o += log rf + log rb
                        lr = mp.tile([128, BL], F32, tag="lr")
                        nc.scalar.activation(lr[64:65, :], rcf[64:65, :], AF.Ln)
                        nc.vector.tensor_add(out=o_acc, in0=o_acc, in1=lr[64:65, :])
                        lr2 = mp.tile([128, BL], F32, tag="lr")
                        nc.scalar.activation(lr2[64:65, :], rcb[64:65, :], AF.Ln)
                        nc.vector.tensor_add(out=o_acc, in0=o_acc, in1=lr2[64:65, :])
                        # 1/r, broadcast to 64 partitions via K=1 ones matmul
                        rinv = mp.tile([128, BL], F32, tag="rinv")
                        nc.vector.reciprocal(out=rinv[64:65, :], in_=rcf[64:65, :])
                        rinv2 = mp.tile([128, BL], F32, tag="rinv")
                        nc.vector.reciprocal(out=rinv2[64:65, :], in_=rcb[64:65, :])
                        r2t = pp2.tile([128, BL], F32, tag="r2", bufs=1)
                        nc.tensor.matmul(out=r2t[0:64, :], lhsT=ones[64:65, 0:64], rhs=rinv[64:65, :])
                        nc.tensor.matmul(out=r2t[64:128, :], lhsT=ones[64:65, 0:64], rhs=rinv2[64:65, :])
                        pending = (s + 2, r2t)

            # ---- final combine: Z = q_255 . (E h_256) ----
            psf = pp.tile([128, BL], F32, tag="ps")
            nc.tensor.matmul(
                out=psf[0:64, :], lhsT=wsb[64:128, :], rhs=qh[64:128, :], tile_position=(64, 0)
            )
            ztmp = mp.tile([128, BL], F32)
            nc.vector.tensor_tensor(out=ztmp[0:64, :], in0=psf[0:64, :], in1=qh[0:64, :], op=ALU.mult)
            zc = pp2.tile([128, BL], F32, tag="rc")
            nc.tensor.matmul(out=zc[64:65, :], lhsT=ones[0:64, 0:1], rhs=ztmp[0:64, :])
            lz = mp.tile([128, BL], F32)
            nc.scalar.activation(lz[64:65, :], zc[64:65, :], AF.Ln)
            nc.vector.tensor_add(out=lz[64:65, :], in0=lz[64:65, :], in1=o_acc)
            nc.sync.dma_start(out=out_logz[:], in_=lz[64:65, :])

            # ---- gold total: diag(G2) + sum(tvals) ----
            dtile = cpool.tile([128, 128], F32)
            nc.vector.tensor_tensor(out=dtile[:], in0=g2[:], in1=idt[:], op=ALU.mult)
            gold_e = cpool.tile([BL, 1], F32)
            nc.vector.reduce_sum(gold_e[:], dtile[:], axis=mybir.AxisListType.X)
            gold_t = cpool.tile([BL, 1], F32)
            nc.vector.reduce_sum(gold_t[:], tvt[:], axis=mybir.AxisListType.X)
            gold = cpool.tile([BL, 1], F32)
            nc.vector.tensor_add(out=gold[:], in0=gold_e[:], in1=gold_t[:])
            nc.sync.dma_start(out=out_gold[:], in_=gold[:, 0:1])
    nc.finalize()
    return nc


def kernel(**inputs) -> np.ndarray:
    global LAST_RESULTS
    logits = np.asarray(inputs["logits"], dtype=np.float32)  # [1024, 512, 64]
    tags = np.asarray(inputs["tags"]).astype(np.int64)  # [1024, 512]
    trans = np.asarray(inputs["trans_m"], dtype=np.float32)  # [64, 64]
    # mask is all-ones by construction (spec fill=ones); under all-ones the
    # reference's mask terms are identities, so it is not used here.

    E = np.exp(trans).astype(np.float32)
    kappa = float(np.log(np.exp(trans.astype(np.float64)).sum(axis=0)).mean())
    wstat = np.concatenate([E, E.T], axis=0).astype(np.float32)  # [128, 64]
    ident = np.eye(128, dtype=np.float32)

    nc = _build(kappa)

    in_maps = []
    for c in range(NCORES):
        sh = logits[c * BL : (c + 1) * BL]  # [128, 512, 64]
        xt = sh.transpose(1, 2, 0)  # [t, j, b]
        x2 = np.concatenate([xt[0:S], xt[S:L][::-1]], axis=1)  # [256, 128, 128] = [s, p, b]
        # regroup into [chunk, partition, pair-in-chunk*b] contiguous chunks
        x3 = (
            x2.reshape(NCH, CHUNK, 128, BL)
            .transpose(0, 2, 1, 3)
            .reshape(NCH, 128, CHUNK * BL)
        )
        tg = tags[c * BL : (c + 1) * BL]  # [128, 512]
        tgf = tg.T[:S]  # [s, b] for t = s
        tgb = tg.T[::-1][:S]  # [s, b] for t = 511 - s
        ohu_s = np.zeros((S, 128, BL), np.uint8)  # [s, p, b] two-hot columns
        s_g = np.arange(S)[:, None]
        b_g = np.arange(BL)[None, :]
        ohu_s[s_g, tgf, b_g] = 1
        ohu_s[s_g, T + tgb, b_g] = 1
        ohu_c = (
            ohu_s.reshape(NCH, CHUNK, 128, BL)
            .transpose(0, 2, 1, 3)
            .reshape(NCH, 128, CHUNK * BL)
        )
        tvals = np.zeros((BL, L), np.float32)
        tvals[:, : L - 1] = trans[tg[:, :-1], tg[:, 1:]]
        in_maps.append(
            {
                "x": np.ascontiguousarray(x3, dtype=np.float32),
                "wstat": wstat,
                "ohu": np.ascontiguousarray(ohu_c),
                "ident": ident,
                "tvals": tvals,
            }
        )

    res = run_bass_kernel_spmd(
        nc,
        in_maps,
        list(range(NCORES)),
        trace=bool(int(os.environ.get("CRF_TRACE", "0"))),
    )
    LAST_RESULTS = res

    out = np.empty((B,), np.float32)
    kc = np.float32(L * kappa)
    for c in range(NCORES):
        logz = np.asarray(res.results[c]["logz"], np.float32).reshape(BL)
        gold = np.asarray(res.results[c]["gold"], np.float32).reshape(BL)
        out[c * BL : (c + 1) * BL] = logz + kc - gold
    return out


# revision 24
# speedup vs baseline: 3.4309x; 1.1377x over previous
"""CRF negative-log-likelihood loss on 8 Trainium2 NeuronCores.

Math (per batch row b, reference semantics, mask == all-ones):
    loss[b] = logsumexp_scan(logits[b], trans) - gold_score(logits[b], tags[b], trans)

Device algorithm (linear domain):
    E = exp(trans), g_t = exp(emit_t - kappa)
    alpha-exp recurrence:  q_t = (E^T q_{t-1}) * g_t     (64x64 matmul + eltwise mul)
    Z = v0 . (A_1 ... A_511) . 1  with A_t = E diag(g_t)  is split into a forward
    scan from t=0 and a backward scan from t=511 that meet in the middle; both
    run in one block-diagonal [128,128] matmul per step (partitions 0-63 fwd,
    64-127 bwd).  Per-row sum-renormalization every KR steps keeps fp32 in
    range; the applied 1/r vectors are stored and logged once at the end.
    Partition-dim reductions/broadcasts are matmuls against ones vectors.
    Gold emission score: host-built one-hot tiles (bf16), accumulated against
    bf16 copies of the raw emission tiles with PSUM-accumulating matmuls; the
    diagonal of the accumulated [128,128] result is the per-row emission score.
    Gold transition score: host-indexed trans[tag_t, tag_{t+1}] table (tiny
    tags-only preprocessing), reduced on device.

Sharding: pure data parallel, batch 1024 -> 8 cores x 128 rows.
"""
import os
import sys

import numpy as np

sys.path.insert(0, "/opt/trn_rl_repo")

from concourse import bacc, bass, mybir, tile  # noqa: E402
from concourse.bass_utils import run_bass_kernel_spmd  # noqa: E402

F32 = mybir.dt.float32
BF16 = mybir.dt.bfloat16
AF = mybir.ActivationFunctionType
ALU = mybir.AluOpType

B, L, T = 1024, 512, 64
NCORES = 8
BL = B // NCORES  # 128 batch rows per core
S = L // 2  # 256 fwd/bwd step-pairs
KR = 32  # renorm every KR pairs (fp32 range stays within ~1e8)
CHUNK = 8  # pairs per DMA/exp chunk -> [128, 1024] tiles
NCH = S // CHUNK
NREN = len([s for s in range(1, S) if s % KR == 0 and s + 2 < S])  # renorms/chain

LAST_RESULTS = None  # BassKernelResults of the most recent run (for test harness)


def _build(kappa: float) -> bass.Bass:
    nc = bacc.Bacc("TRN2", target_bir_lowering=False, debug=False, num_devices=NCORES)
    # x layout: [chunk, partition, pair-within-chunk * b] — each chunk tile is
    # one contiguous-per-partition [128, CHUNK*BL] DMA.
    x = nc.dram_tensor("x", [NCH, 128, CHUNK * BL], F32, kind="ExternalInput")
    w2d = nc.dram_tensor("w2d", [128, 128], F32, kind="ExternalInput")  # blockdiag(E, E^T)
    wfin = nc.dram_tensor("wfin", [128, T], F32, kind="ExternalInput")  # rows 64-127: E^T
    # one-hot of the gold tag per (partition-half, pair, b), bf16
    ohu = nc.dram_tensor("ohu", [NCH, 128, CHUNK * BL], BF16, kind="ExternalInput")
    ident = nc.dram_tensor("ident", [128, 128], F32, kind="ExternalInput")
    tvals = nc.dram_tensor("tvals", [BL, L], F32, kind="ExternalInput")
    out_logz = nc.dram_tensor("logz", [BL], F32, kind="ExternalOutput")
    out_gold = nc.dram_tensor("gold", [BL], F32, kind="ExternalOutput")

    RW = 2 * NREN * BL  # rinv store width (fwd+bwd slots)

    with tile.TileContext(nc) as tc:
        with (
            tc.tile_pool(name="cpool", bufs=1) as cpool,
            tc.tile_pool(name="rawp", bufs=3) as rawp,
            tc.tile_pool(name="gp", bufs=3) as gp,
            tc.tile_pool(name="ohp", bufs=3) as ohp,
            tc.tile_pool(name="qp", bufs=6) as qp,
            tc.tile_pool(name="mp", bufs=2) as mp,
            tc.tile_pool(name="pp", bufs=4, space="PSUM") as pp,
            tc.tile_pool(name="pp2", bufs=2, space="PSUM") as pp2,
            tc.tile_pool(name="ppg", bufs=1, space="PSUM") as ppg,
        ):
            # ---- constants ----
            w2 = cpool.tile([128, 128], F32)
            nc.sync.dma_start(out=w2[:], in_=w2d[:])
            wf = cpool.tile([128, T], F32)
            nc.sync.dma_start(out=wf[:], in_=wfin[:])
            ones = cpool.tile([128, T], F32)
            nc.gpsimd.memset(ones[:], 1.0)
            kbias = cpool.tile([128, 1], F32)  # per-partition bias = -kappa for Exp
            nc.vector.memset(kbias[:], -kappa)
            idt = cpool.tile([128, 128], F32)
            nc.sync.dma_start(out=idt[:], in_=ident[:])
            tvt = cpool.tile([BL, L], F32)
            nc.sync.dma_start(out=tvt[:], in_=tvals[:])
            rstore = cpool.tile([128, RW], F32)  # row 64: applied rinv values
            nc.vector.memset(rstore[:], 1.0)

            g2 = ppg.tile([128, 128], F32)  # gold-emission accumulator

            # ---- fwd/bwd scan + gold-emission accumulation ----
            qh = None  # [128, BL]: rows 0-63 fwd state q, rows 64-127 bwd state h
            pending = None  # (apply_at_pair, R2 psum tile) for lagged renorm
            nren = 0
            for c in range(NCH):
                raw = rawp.tile([128, CHUNK * BL], F32)
                nc.sync.dma_start(out=raw[:], in_=x[c])
                g = gp.tile([128, CHUNK * BL], F32)
                nc.scalar.activation(g[:], raw[:], AF.Exp, bias=kbias[:])
                # bf16 copies for the gold matmuls
                rawb = ohp.tile([128, CHUNK * BL], BF16, tag="rawb")
                nc.scalar.copy(out=rawb[:], in_=raw[:])
                oh = ohp.tile([128, CHUNK * BL], BF16, tag="oh")
                nc.sync.dma_start(out=oh[:], in_=ohu[c])
                for k in range(CHUNK):
                    s = c * CHUNK + k
                    sl = slice(k * BL, (k + 1) * BL)
                    nc.tensor.matmul(
                        out=g2[:],
                        lhsT=oh[:, sl],
                        rhs=rawb[:, sl],
                        start=(s == 0),
                        stop=(s == S - 1),
                        skip_group_check=True,
                    )
                    gt = g[:, sl]
                    if s == 0:
                        qh = gt  # q_0 = g_0 (fwd), h_511 = g_511 (bwd)
                        continue
                    ps = pp.tile([128, BL], F32, tag="ps")
                    nc.tensor.matmul(out=ps[:], lhsT=w2[:], rhs=qh[:])
                    nqh = qp.tile([128, BL], F32, tag="qh")
                    nc.vector.tensor_tensor(out=nqh[:], in0=ps[:], in1=gt, op=ALU.mult)
                    qh = nqh
                    if pending is not None and pending[0] == s:
                        r2 = pending[1]
                        nqh2 = qp.tile([128, BL], F32, tag="qh")
                        nc.vector.tensor_tensor(out=nqh2[:], in0=qh[:], in1=r2[:], op=ALU.mult)
                        qh = nqh2
                        pending = None
                    if s % KR == 0 and s + 2 < S:
                        # colsums of both chains (partition-dim reduction via ones matmul)
                        rcf = pp2.tile([128, BL], F32, tag="rc")
                        rcb = pp2.tile([128, BL], F32, tag="rc")
                        nc.tensor.matmul(out=rcf[64:65, :], lhsT=ones[0:64, 0:1], rhs=qh[0:64, :])
                        nc.tensor.matmul(out=rcb[64:65, :], lhsT=ones[64:128, 0:1], rhs=qh[64:128, :])
                        # 1/r into the persistent store (logged once at the end),
                        # broadcast to 64 partitions via K=1 ones matmul
                        rif = rstore[64:65, 2 * nren * BL : (2 * nren + 1) * BL]
                        rib = rstore[64:65, (2 * nren + 1) * BL : (2 * nren + 2) * BL]
                        nc.vector.reciprocal(out=rif, in_=rcf[64:65, :])
                        nc.vector.reciprocal(out=rib, in_=rcb[64:65, :])
                        r2t = pp2.tile([128, BL], F32, tag="r2", bufs=1)
                        nc.tensor.matmul(out=r2t[0:64, :], lhsT=ones[64:65, 0:64], rhs=rif)
                        nc.tensor.matmul(out=r2t[64:128, :], lhsT=ones[64:65, 0:64], rhs=rib)
                        pending = (s + 2, r2t)
                        nren += 1

            # ---- final combine: Z = q_255 . (E h_256) ----
            psf = pp.tile([128, BL], F32, tag="ps")
            nc.tensor.matmul(
                out=psf[0:64, :], lhsT=wf[64:128, :], rhs=qh[64:128, :], tile_position=(64, 0)
            )
            ztmp = mp.tile([128, BL], F32)
            nc.vector.tensor_tensor(out=ztmp[0:64, :], in0=psf[0:64, :], in1=qh[0:64, :], op=ALU.mult)
            zc = pp2.tile([128, BL], F32, tag="rc")
            nc.tensor.matmul(out=zc[64:65, :], lhsT=ones[0:64, 0:1], rhs=ztmp[0:64, :])
            # logZ = Ln(Z) - sum_i ln(rinv_i)   (minus kappa terms added on host)
            lnr = cpool.tile([128, RW], F32)
            nc.scalar.activation(lnr[64:65, :], rstore[64:65, :], AF.Ln)
            slr = cpool.tile([128, 1 * BL], F32)
            nc.vector.reduce_sum(
                slr[64:65, :],
                lnr[64:65, :].rearrange("p (i b) -> p b i", i=2 * NREN),
                axis=mybir.AxisListType.X,
            )
            lz = mp.tile([128, BL], F32)
            nc.scalar.activation(lz[64:65, :], zc[64:65, :], AF.Ln)
            lzf = cpool.tile([128, BL], F32)
            nc.vector.tensor_sub(out=lzf[64:65, :], in0=lz[64:65, :], in1=slr[64:65, :])
            nc.sync.dma_start(out=out_logz[:], in_=lzf[64:65, :])

            # ---- gold total: diag(G2) + sum(tvals) ----
            dtile = cpool.tile([128, 128], F32)
            nc.vector.tensor_tensor(out=dtile[:], in0=g2[:], in1=idt[:], op=ALU.mult)
            gold_e = cpool.tile([BL, 1], F32)
            nc.vector.reduce_sum(gold_e[:], dtile[:], axis=mybir.AxisListType.X)
            gold_t = cpool.tile([BL, 1], F32)
            nc.vector.reduce_sum(gold_t[:], tvt[:], axis=mybir.AxisListType.X)
            gold = cpool.tile([BL, 1], F32)
            nc.vector.tensor_add(out=gold[:], in0=gold_e[:], in1=gold_t[:])
            nc.sync.dma_start(out=out_gold[:], in_=gold[:, 0:1])
    nc.finalize()
    return nc


def kernel(**inputs) -> np.ndarray:
    global LAST_RESULTS
    logits = np.asarray(inputs["logits"], dtype=np.float32)  # [1024, 512, 64]
    tags = np.asarray(inputs["tags"]).astype(np.int64)  # [1024, 512]
    trans = np.asarray(inputs["trans_m"], dtype=np.float32)  # [64, 64]
    # mask is all-ones by construction (spec fill=ones); under all-ones the
    # reference's mask terms are identities, so it is not used here.

    E = np.exp(trans).astype(np.float32)
    kappa = float(np.log(np.exp(trans.astype(np.float64)).sum(axis=0)).mean())
    w2d = np.zeros((128, 128), np.float32)
    w2d[0:T, 0:T] = E
    w2d[T:128, T:128] = E.T
    wfin = np.concatenate([E, E.T], axis=0).astype(np.float32)  # [128, 64]
    ident = np.eye(128, dtype=np.float32)
    bf16 = mybir.dt.np(BF16)

    nc = _build(kappa)

    in_maps = []
    for c in range(NCORES):
        sh = logits[c * BL : (c + 1) * BL]  # [128, 512, 64]
        xt = sh.transpose(1, 2, 0)  # [t, j, b]
        x2 = np.concatenate([xt[0:S], xt[S:L][::-1]], axis=1)  # [256, 128, 128] = [s, p, b]
        # regroup into [chunk, partition, pair-in-chunk*b] contiguous chunks
        x3 = (
            x2.reshape(NCH, CHUNK, 128, BL)
            .transpose(0, 2, 1, 3)
            .reshape(NCH, 128, CHUNK * BL)
        )
        tg = tags[c * BL : (c + 1) * BL]  # [128, 512]
        tgf = tg.T[:S]  # [s, b] for t = s
        tgb = tg.T[::-1][:S]  # [s, b] for t = 511 - s
        ohu_s = np.zeros((S, 128, BL), bf16)  # [s, p, b] two-hot columns
        s_g = np.arange(S)[:, None]
        b_g = np.arange(BL)[None, :]
        ohu_s[s_g, tgf, b_g] = 1
        ohu_s[s_g, T + tgb, b_g] = 1
        ohu_c = (
            ohu_s.reshape(NCH, CHUNK, 128, BL)
            .transpose(0, 2, 1, 3)
            .reshape(NCH, 128, CHUNK * BL)
        )
        tvals = np.zeros((BL, L), np.float32)
        tvals[:, : L - 1] = trans[tg[:, :-1], tg[:, 1:]]
        in_maps.append(
            {
                "x": np.ascontiguousarray(x3, dtype=np.float32),
                "w2d": w2d,
                "wfin": wfin,
                "ohu": np.ascontiguousarray(ohu_c),
                "ident": ident,
                "tvals": tvals,
            }
        )

    res = run_bass_kernel_spmd(
        nc,
        in_maps,
        list(range(NCORES)),
        trace=bool(int(os.environ.get("CRF_TRACE", "0"))),
    )
    LAST_RESULTS = res

    out = np.empty((B,), np.float32)
    kc = np.float32(L * kappa)
    for c in range(NCORES):
        logz = np.asarray(res.results[c]["logz"], np.float32).reshape(BL)
        gold = np.asarray(res.results[c]["gold"], np.float32).reshape(BL)
        out[c * BL : (c + 1) * BL] = logz + kc - gold
    return out


# revision 31
# speedup vs baseline: 4.3585x; 1.2704x over previous
"""CRF negative-log-likelihood loss on 8 Trainium2 NeuronCores.

Math (per batch row b, reference semantics, mask == all-ones):
    loss[b] = logsumexp_scan(logits[b], trans) - gold_score(logits[b], tags[b], trans)

Device algorithm (linear domain):
    E = exp(trans), g_t = exp(emit_t - kappa)
    alpha-exp recurrence:  q_t = (E^T q_{t-1}) * g_t     (64x64 matmul + eltwise mul)
    Z = v0 . (A_1 ... A_511) . 1  with A_t = E diag(g_t)  is split into a forward
    scan from t=0 and a backward scan from t=511 that meet in the middle; both
    run in one block-diagonal [128,128] matmul per step (partitions 0-63 fwd,
    64-127 bwd).  Per-row sum-renormalization every KR steps keeps fp32 in
    range; the applied 1/r vectors are stored and logged once at the end.
    Partition-dim reductions/broadcasts are matmuls against ones vectors.
    Gold emission score: host-built one-hot tiles (bf16), accumulated against
    bf16 copies of the raw emission tiles with PSUM-accumulating matmuls; the
    diagonal of the accumulated [128,128] result is the per-row emission score.
    Gold transition score: host-indexed trans[tag_t, tag_{t+1}] table (tiny
    tags-only preprocessing), reduced on device.

Sharding: pure data parallel, batch 1024 -> 8 cores x 128 rows.
"""
import os
import sys

import numpy as np

sys.path.insert(0, "/opt/trn_rl_repo")

from concourse import bacc, bass, mybir, tile  # noqa: E402
from concourse.bass_utils import run_bass_kernel_spmd  # noqa: E402

F32 = mybir.dt.float32
F32R = mybir.dt.float32r
BF16 = mybir.dt.bfloat16
AF = mybir.ActivationFunctionType
ALU = mybir.AluOpType

B, L, T = 1024, 512, 64
NCORES = 8
BL = B // NCORES  # 128 batch rows per core
S = L // 2  # 256 fwd/bwd step-pairs
KR = 32  # renorm every KR pairs (fp32 range stays within ~1e8)
CHUNK = 8  # pairs per DMA/exp chunk -> [128, 1024] tiles
NCH = S // CHUNK
NREN = len([s for s in range(1, S) if s % KR == 0 and s + 2 < S])  # renorms/chain

LAST_RESULTS = None  # BassKernelResults of the most recent run (for test harness)


def _build(kappa: float) -> bass.Bass:
    nc = bacc.Bacc("TRN2", target_bir_lowering=False, debug=False, num_devices=NCORES)
    # x layout: [chunk, partition, pair-within-chunk * b] — each chunk tile is
    # one contiguous-per-partition [128, CHUNK*BL] DMA.
    x = nc.dram_tensor("x", [NCH, 128, CHUNK * BL], F32, kind="ExternalInput")
    w2d = nc.dram_tensor("w2d", [128, 128], F32R, kind="ExternalInput")  # blockdiag(E, E^T)
    wfin = nc.dram_tensor("wfin", [128, T], F32, kind="ExternalInput")  # rows 64-127: E^T
    # one-hot of the gold tag per (partition-half, pair, b), bf16
    ohu = nc.dram_tensor("ohu", [NCH, 128, CHUNK * BL], BF16, kind="ExternalInput")
    ident = nc.dram_tensor("ident", [128, 128], F32, kind="ExternalInput")
    onesd = nc.dram_tensor("onesd", [128, T], F32R, kind="ExternalInput")
    tvals = nc.dram_tensor("tvals", [BL, L], F32, kind="ExternalInput")
    out_logz = nc.dram_tensor("logz", [BL], F32, kind="ExternalOutput")
    out_gold = nc.dram_tensor("gold", [BL], F32, kind="ExternalOutput")

    RW = 2 * NREN * BL  # rinv store width (fwd+bwd slots)

    with tile.TileContext(nc) as tc:
        with (
            tc.tile_pool(name="cpool", bufs=1) as cpool,
            tc.tile_pool(name="rawp", bufs=4) as rawp,
            tc.tile_pool(name="gp", bufs=4) as gp,
            tc.tile_pool(name="ohp", bufs=3) as ohp,
            tc.tile_pool(name="qp", bufs=6) as qp,
            tc.tile_pool(name="mp", bufs=2) as mp,
            tc.tile_pool(name="pp", bufs=4, space="PSUM") as pp,
            tc.tile_pool(name="pp2", bufs=2, space="PSUM") as pp2,
            tc.tile_pool(name="ppg", bufs=1, space="PSUM") as ppg,
        ):
            # ---- constants ----
            w2 = cpool.tile([128, 128], F32R)
            nc.sync.dma_start(out=w2[:], in_=w2d[:])
            wf = cpool.tile([128, T], F32)
            nc.sync.dma_start(out=wf[:], in_=wfin[:])
            ones = cpool.tile([128, T], F32R)
            nc.sync.dma_start(out=ones[:], in_=onesd[:])
            kbias = cpool.tile([128, 1], F32)  # per-partition bias = -kappa for Exp
            nc.vector.memset(kbias[:], -kappa)
            idt = cpool.tile([128, 128], F32)
            nc.sync.dma_start(out=idt[:], in_=ident[:])
            tvt = cpool.tile([BL, L], F32)
            nc.sync.dma_start(out=tvt[:], in_=tvals[:])
            rstore = cpool.tile([128, RW], F32)  # row 64: applied rinv values
            nc.vector.memset(rstore[:], 1.0)

            g2 = ppg.tile([128, 128], F32)  # gold-emission accumulator

            # ---- fwd/bwd scan + gold-emission accumulation ----
            qh = None  # [128, BL]: rows 0-63 fwd state q, rows 64-127 bwd state h
            pending = None  # (apply_at_pair, R2 psum tile) for lagged renorm
            nren = 0
            for c in range(NCH):
                raw = rawp.tile([128, CHUNK * BL], F32)
                nc.sync.dma_start(out=raw[:], in_=x[c])
                g = gp.tile([128, CHUNK * BL], F32R)
                half = CHUNK * BL // 2
                nc.scalar.activation(g[:, 0:half], raw[:, 0:half], AF.Exp, bias=kbias[:])
                nc.scalar.activation(g[:, half:], raw[:, half:], AF.Exp, bias=kbias[:])
                # bf16 copies for the gold matmuls
                rawb = ohp.tile([128, CHUNK * BL], BF16, tag="rawb")
                nc.scalar.copy(out=rawb[:], in_=raw[:])
                oh = ohp.tile([128, CHUNK * BL], BF16, tag="oh")
                nc.sync.dma_start(out=oh[:], in_=ohu[c])
                for k in range(CHUNK):
                    s = c * CHUNK + k
                    sl = slice(k * BL, (k + 1) * BL)
                    nc.tensor.matmul(
                        out=g2[:],
                        lhsT=oh[:, sl],
                        rhs=rawb[:, sl],
                        start=(s == 0),
                        stop=(s == S - 1),
                        skip_group_check=True,
                    )
                    gt = g[:, sl]
                    if s == 0:
                        qh = gt  # q_0 = g_0 (fwd), h_511 = g_511 (bwd)
                        continue
                    ps = pp.tile([128, BL], F32, tag="ps")
                    nc.tensor.matmul(out=ps[:], lhsT=w2[:], rhs=qh[:])
                    nqh = qp.tile([128, BL], F32R, tag="qh")
                    nc.vector.tensor_tensor(out=nqh[:], in0=ps[:], in1=gt, op=ALU.mult)
                    qh = nqh
                    if pending is not None and pending[0] == s:
                        r2 = pending[1]
                        nqh2 = qp.tile([128, BL], F32R, tag="qh")
                        nc.vector.tensor_tensor(out=nqh2[:], in0=qh[:], in1=r2[:], op=ALU.mult)
                        qh = nqh2
                        pending = None
                    if s % KR == 0 and s + 2 < S:
                        # colsums of both chains (partition-dim reduction via ones matmul)
                        rcf = pp2.tile([128, BL], F32, tag="rc")
                        rcb = pp2.tile([128, BL], F32, tag="rc")
                        nc.tensor.matmul(out=rcf[64:65, :], lhsT=ones[0:64, 0:1].bitcast(F32), rhs=qh[0:64, :].bitcast(F32))
                        nc.tensor.matmul(out=rcb[64:65, :], lhsT=ones[64:128, 0:1].bitcast(F32), rhs=qh[64:128, :].bitcast(F32))
                        # 1/r into the persistent store (logged once at the end),
                        # broadcast to 64 partitions via K=1 ones matmul
                        rif = rstore[64:65, 2 * nren * BL : (2 * nren + 1) * BL]
                        rib = rstore[64:65, (2 * nren + 1) * BL : (2 * nren + 2) * BL]
                        nc.vector.reciprocal(out=rif, in_=rcf[64:65, :])
                        nc.vector.reciprocal(out=rib, in_=rcb[64:65, :])
                        r2t = pp2.tile([128, BL], F32, tag="r2", bufs=1)
                        nc.tensor.matmul(out=r2t[0:64, :], lhsT=ones[64:65, 0:64].bitcast(F32), rhs=rif)
                        nc.tensor.matmul(out=r2t[64:128, :], lhsT=ones[64:65, 0:64].bitcast(F32), rhs=rib)
                        pending = (s + 2, r2t)
                        nren += 1

            # ---- final combine: Z = q_255 . (E h_256) ----
            psf = pp.tile([128, BL], F32, tag="ps")
            nc.tensor.matmul(
                out=psf[0:64, :], lhsT=wf[64:128, :].bitcast(F32), rhs=qh[64:128, :].bitcast(F32), tile_position=(64, 0)
            )
            ztmp = mp.tile([128, BL], F32)
            nc.vector.tensor_tensor(out=ztmp[0:64, :], in0=psf[0:64, :], in1=qh[0:64, :], op=ALU.mult)
            zc = pp2.tile([128, BL], F32, tag="rc")
            nc.tensor.matmul(out=zc[64:65, :], lhsT=ones[0:64, 0:1].bitcast(F32), rhs=ztmp[0:64, :])
            # logZ = Ln(Z) - sum_i ln(rinv_i)   (minus kappa terms added on host)
            lnr = cpool.tile([128, RW], F32)
            nc.scalar.activation(lnr[64:65, :], rstore[64:65, :], AF.Ln)
            slr = cpool.tile([128, 1 * BL], F32)
            nc.vector.reduce_sum(
                slr[64:65, :],
                lnr[64:65, :].rearrange("p (i b) -> p b i", i=2 * NREN),
                axis=mybir.AxisListType.X,
            )
            lz = mp.tile([128, BL], F32)
            nc.scalar.activation(lz[64:65, :], zc[64:65, :], AF.Ln)
            lzf = cpool.tile([128, BL], F32)
            nc.vector.tensor_sub(out=lzf[64:65, :], in0=lz[64:65, :], in1=slr[64:65, :])
            nc.sync.dma_start(out=out_logz[:], in_=lzf[64:65, :])

            # ---- gold total: diag(G2) + sum(tvals) ----
            dtile = cpool.tile([128, 128], F32)
            nc.vector.tensor_tensor(out=dtile[:], in0=g2[:], in1=idt[:], op=ALU.mult)
            gold_e = cpool.tile([BL, 1], F32)
            nc.vector.reduce_sum(gold_e[:], dtile[:], axis=mybir.AxisListType.X)
            gold_t = cpool.tile([BL, 1], F32)
            nc.vector.reduce_sum(gold_t[:], tvt[:], axis=mybir.AxisListType.X)
            gold = cpool.tile([BL, 1], F32)
            nc.vector.tensor_add(out=gold[:], in0=gold_e[:], in1=gold_t[:])
            nc.sync.dma_start(out=out_gold[:], in_=gold[:, 0:1])
    nc.finalize()
    return nc


def kernel(**inputs) -> np.ndarray:
    global LAST_RESULTS
    logits = np.asarray(inputs["logits"], dtype=np.float32)  # [1024, 512, 64]
    tags = np.asarray(inputs["tags"]).astype(np.int64)  # [1024, 512]
    trans = np.asarray(inputs["trans_m"], dtype=np.float32)  # [64, 64]
    # mask is all-ones by construction (spec fill=ones); under all-ones the
    # reference's mask terms are identities, so it is not used here.

    E = np.exp(trans).astype(np.float32)
    kappa = float(np.log(np.exp(trans.astype(np.float64)).sum(axis=0)).mean())
    w2d = np.zeros((128, 128), np.float32)
    w2d[0:T, 0:T] = E
    w2d[T:128, T:128] = E.T
    wfin = np.concatenate([E, E.T], axis=0).astype(np.float32)  # [128, 64]
    ident = np.eye(128, dtype=np.float32)
    bf16 = mybir.dt.np(BF16)

    nc = _build(kappa)

    in_maps = []
    for c in range(NCORES):
        sh = logits[c * BL : (c + 1) * BL]  # [128, 512, 64]
        xt = sh.transpose(1, 2, 0)  # [t, j, b]
        x2 = np.concatenate([xt[0:S], xt[S:L][::-1]], axis=1)  # [256, 128, 128] = [s, p, b]
        # regroup into [chunk, partition, pair-in-chunk*b] contiguous chunks
        x3 = (
            x2.reshape(NCH, CHUNK, 128, BL)
            .transpose(0, 2, 1, 3)
            .reshape(NCH, 128, CHUNK * BL)
        )
        tg = tags[c * BL : (c + 1) * BL]  # [128, 512]
        tgf = tg.T[:S]  # [s, b] for t = s
        tgb = tg.T[::-1][:S]  # [s, b] for t = 511 - s
        ohu_s = np.zeros((S, 128, BL), bf16)  # [s, p, b] two-hot columns
        s_g = np.arange(S)[:, None]
        b_g = np.arange(BL)[None, :]
        ohu_s[s_g, tgf, b_g] = 1
        ohu_s[s_g, T + tgb, b_g] = 1
        ohu_c = (
            ohu_s.reshape(NCH, CHUNK, 128, BL)
            .transpose(0, 2, 1, 3)
            .reshape(NCH, 128, CHUNK * BL)
        )
        tvals = np.zeros((BL, L), np.float32)
        tvals[:, : L - 1] = trans[tg[:, :-1], tg[:, 1:]]
        in_maps.append(
            {
                "x": np.ascontiguousarray(x3, dtype=np.float32),
                "w2d": w2d,
                "wfin": wfin,
                "ohu": np.ascontiguousarray(ohu_c),
                "ident": ident,
                "onesd": np.ones((128, T), np.float32),
                "tvals": tvals,
            }
        )

    res = run_bass_kernel_spmd(
        nc,
        in_maps,
        list(range(NCORES)),
        trace=bool(int(os.environ.get("CRF_TRACE", "0"))),
    )
    LAST_RESULTS = res

    out = np.empty((B,), np.float32)
    kc = np.float32(L * kappa)
    for c in range(NCORES):
        logz = np.asarray(res.results[c]["logz"], np.float32).reshape(BL)
        gold = np.asarray(res.results[c]["gold"], np.float32).reshape(BL)
        out[c * BL : (c + 1) * BL] = logz + kc - gold
    return out
